# revision 14
# baseline (speedup 1.0000x reference)
"""Causal self-attention on 8 trn2 NeuronCores.

Sharding: core c -> (batch b = c//2, head-group hg = c%2 of 8 heads).
Each core computes, for its batch and its 8 heads:
  qT,kT = (x[b] @ Wqk_shard).T        (q pre-scaled by 1/sqrt(hd))
  V     = x[b] @ Wv_shard
  S^T   = kT_h.T @ qT_h  per head     (s on partitions, t on free dim)
  P^T   = exp(S^T), diag tiles masked multiplicatively on DVE
  yT    = V_aug.T @ P^T               (V carries a ones column -> row 64 = softmax denom)
  out_partial = y_local @ Wout_rows   ([T, E] fp32 partial sum)
Host: out[b] = partial[2b] + partial[2b+1] + b_out.

Heads are processed in PAIRS: the two heads of a pair occupy SBUF
partitions 0-63 / 64-127 of the shared qT/kT tiles, so their S^T matmuls
(contraction K=64) map to row-tiled PE positions (0,0) and (64,0) and run
CONCURRENTLY in the 128x128 array (2x), with each tile's LDWEIGHTS
overlapping the other tile's matmul.  The pair's two S outputs land in
one 2-bank PSUM tile so a single strided ACT exp serves both heads.

The attention stream is ACT(exp)-paced (~1us per s-tile vs ~0.7us of PE
work), so the projections for the NEXT chunk, the output projection of
the PREVIOUS chunk, and the softmax-divide tails are chopped into small
"filler units" and emitted between attention groups: the in-order PE
queue then always has independent matmuls to chew on while exp runs.

Each pair's two softmax denominators are batched on partitions 0/64 of
one tile so a single DVE reciprocal serves both heads; the causal-mask
multiply runs on GPSIMD so the DVE's strict FIFO (which parks behind
that 3.3us reciprocal at pair ends) never gates the AV matmuls.

All matmul inputs fp16, PSUM accumulation fp32. x is pre-transposed and
pre-cast on host so no on-chip transpose is needed.
"""

import numpy as np

B, T, E, H, HD = 4, 2048, 1024, 16, 64
HPC = 8            # heads per core
DL = HPC * HD      # 512 local y dims per core
NT = T // 512      # 4 t-chunks of 512
NS = T // 128      # 16 s-tiles of 128
NE = E // 128      # 8 e-tiles

_CACHE = {}


def _make_tc_class():
    """TileContext whose tail drain splits sem waits across single-wait NOPs.

    The walrus build in this container rejects instructions carrying more
    than a couple of sync waits ("Too many sync wait commands" on the Tile
    tail Drain), so emit one NOP per logical proc, each with one wait.
    """
    import concourse.tile as tile
    from concourse.vector_clock import ScopedClock, VectorClock

    class TC(tile.TileContext):
        def _drain_and_barrier(self, tick_clock, wait_clock):
            gc = tick_clock.global_clock
            n = len(gc)
            for i in range(n):
                if gc[i] > 0:
                    vc = VectorClock([0] * n)
                    vc.require_at_least(i, gc[i])
                    nop = self.nc.sync.nop(nofuse=True)
                    wait_clock.add_sem_waits(nop.ins, ScopedClock({None: vc}))
            self.nc.sync.drain()
            self.nc.all_engine_barrier()
            assert self.sems is not None
            popped = self.nc._tile_sem_poison_stack.pop()
            assert popped is self._sem_poison
            self.nc.clear_and_free_semaphores(
                list(self.sems.allocated().values())
            )
            self.nc.all_engine_barrier()

    return TC


def _split_excess_waits(nc, max_waits=2):
    """Walrus in this container caps sem waits per instruction; hoist any
    excess waits onto fresh same-engine NOPs inserted just before."""
    import concourse.mybir as mybir

    import concourse.mybir as _mybir

    n = 0
    for f in nc.m.functions:
        for bb in f.blocks:
            insts = bb.instructions
            out = []
            for inst in insts:
                si = inst.sync_info
                cap = (
                    1
                    if inst.engine == _mybir.EngineType.Activation
                    else max_waits
                )
                if si is not None and len(si.on_wait) > cap:
                    w = list(si.on_wait)
                    excess, keep = w[:-cap], w[-cap:]
                    for k in range(0, len(excess), cap):
                        nop = mybir.InstNoOp(
                            name=f"I-splitw-{n}", ins=[], outs=[]
                        )
                        n += 1
                        nop.engine = inst.engine
                        nop.sync_info = mybir.SyncInfo(
                            on_wait=excess[k:k + cap], on_update=[]
                        )
                        out.append(nop)
                    inst.sync_info = mybir.SyncInfo(
                        on_wait=keep, on_update=si.on_update
                    )
                out.append(inst)
            if n:
                bb.instructions = out
    return nc


def _build():
    import concourse.bass as bass
    import concourse.mybir as mybir

    dt = mybir.dt
    f16, f32 = dt.float16, dt.float32
    AF = mybir.ActivationFunctionType

    nc = bass.Bass()
    xt = nc.declare_dram_parameter("xt", [E, T], f16, isOutput=False)
    wqk = nc.declare_dram_parameter("wqk", [E, 1024], f16, isOutput=False)
    bqk = nc.declare_dram_parameter("bqk", [128, 8], f32, isOutput=False)
    wv = nc.declare_dram_parameter("wv", [E, 512], f16, isOutput=False)
    bv = nc.declare_dram_parameter("bv", [1, 512], f16, isOutput=False)
    wo = nc.declare_dram_parameter("wo", [DL, E], f16, isOutput=False)
    # mask2[i, k, j] = 1 if j >= i else 0 (multiplicative causal mask,
    # duplicated along k so one strided DVE multiply covers a head pair)
    mask2 = nc.declare_dram_parameter("mask2", [128, 2, 128], f16,
                                      isOutput=False)
    ones1 = nc.declare_dram_parameter("ones1", [1, 128], f16, isOutput=False)
    out = nc.declare_dram_parameter("out", [T, E], f32, isOutput=True)

    with _make_tc_class()(nc) as tc:
        with (
            tc.tile_pool(name="const", bufs=1) as constp,
            tc.tile_pool(name="xtp", bufs=1) as xtp,
            tc.tile_pool(name="wp", bufs=1) as wp,
            tc.tile_pool(name="qkv", bufs=1) as qkvp,
            tc.tile_pool(name="pt", bufs=6) as ptp,
            tc.tile_pool(name="rec", bufs=2) as recp,
            tc.tile_pool(name="stg", bufs=2) as stgp,
            tc.tile_pool(name="outp", bufs=4) as outp,
            tc.tile_pool(name="psA", bufs=2, space="PSUM") as psA,
            tc.tile_pool(name="psS", bufs=2, space="PSUM") as psS,
            tc.tile_pool(name="psY", bufs=2, space="PSUM") as psY,
        ):
            # ---- constants / weights ----
            bqk_sb = constp.tile([128, 8], f32, tag="bqk")
            nc.sync.dma_start(bqk_sb[:], bqk[:])
            bv_sb = constp.tile([1, 512], f16, tag="bv")
            nc.sync.dma_start(bv_sb[:], bv[:])
            mask2_sb = constp.tile([128, 2, 128], f16, tag="mask2")
            nc.sync.dma_start(mask2_sb[:], mask2[:])
            ones_sb = constp.tile([1, 128], f16, tag="ones1")
            nc.sync.dma_start(ones_sb[:], ones1[:])

            # q-half of wqk + chunk-0 x slices first (16 transfers of 128KB
            # spread across the DMA queues): the jt=0..3 projections of
            # chunk 0 can then start ~6us in instead of waiting for the
            # full 8MB input load.
            xt_sb, wqk_sb, wv_sb, wo_sb = [], [], [], []
            for i in range(NE):
                w_ = wp.tile([128, 1024], f16, tag=f"wqk{i}", name=f"wqk{i}")
                nc.sync.dma_start(
                    w_[:, 0:512], wqk[i * 128:(i + 1) * 128, 0:512]
                )
                wqk_sb.append(w_)
                t_ = xtp.tile([128, T], f16, tag=f"xt{i}", name=f"xt{i}")
                nc.sync.dma_start(
                    t_[:, 0:512], xt[i * 128:(i + 1) * 128, 0:512]
                )
                xt_sb.append(t_)
            for i in range(NE):
                nc.sync.dma_start(
                    wqk_sb[i][:, 512:1024],
                    wqk[i * 128:(i + 1) * 128, 512:1024],
                )
            for i in range(NE):
                t_ = wp.tile([128, 512], f16, tag=f"wv{i}", name=f"wv{i}")
                nc.sync.dma_start(t_[:], wv[i * 128:(i + 1) * 128, :])
                wv_sb.append(t_)
            for c4 in range(1, 4):
                for i in range(NE):
                    nc.sync.dma_start(
                        xt_sb[i][:, c4 * 512:(c4 + 1) * 512],
                        xt[i * 128:(i + 1) * 128, c4 * 512:(c4 + 1) * 512],
                    )
            for i in range(4):
                t_ = wp.tile([128, 1024], f16, tag=f"wo{i}", name=f"wo{i}")
                nc.sync.dma_start(t_[:], wo[i * 128:(i + 1) * 128, :])
                wo_sb.append(t_)

            qt_sb = [qkvp.tile([128, T], f16, tag=f"qt{i}", name=f"qt{i}") for i in range(4)]
            kt_sb = [qkvp.tile([128, T], f16, tag=f"kt{i}", name=f"kt{i}") for i in range(4)]
            yt_sb = [qkvp.tile([128, T], f16, tag=f"yt{i}", name=f"yt{i}") for i in range(4)]
            va_sb = [qkvp.tile([128, 8, 65], f16, tag=f"va{i}", name=f"va{i}") for i in range(NS)]

            # ---- filler units: small emission closures interleaved into
            # the attention stream so the in-order PE never drains ----

            def _qkproj_unit(tcx, jt):
                def u():
                    dest = qt_sb[jt] if jt < 4 else kt_sb[jt - 4]
                    ps = psA.tile([128, 512], f32, tag="psA", name="qks")
                    for et in range(NE):
                        nc.tensor.matmul(
                            ps[:],
                            wqk_sb[et][:, jt * 128:(jt + 1) * 128],
                            xt_sb[et][:, tcx * 512:(tcx + 1) * 512],
                            start=(et == 0),
                            stop=(et == NE - 1),
                        )
                    nc.vector.tensor_scalar_add(
                        dest[:, tcx * 512:(tcx + 1) * 512], ps[:],
                        bqk_sb[:, jt:jt + 1],
                    )
                return u

            def _vproj_unit(st):
                def u():
                    ps = psA.tile([128, 512], f32, tag="psA", name="vps")
                    for et in range(NE):
                        nc.tensor.matmul(
                            ps[:],
                            xt_sb[et][:, st * 128:(st + 1) * 128],
                            wv_sb[et][:],
                            start=(et == 0),
                            stop=False,
                        )
                    # bias row: V += 1 * bv
                    nc.tensor.matmul(
                        ps[:], ones_sb[:], bv_sb[:], start=False, stop=True,
                    )
                    va = va_sb[st]
                    nc.vector.tensor_copy(
                        va[:, :, 0:64],
                        ps[:].rearrange("p (h c) -> p h c", c=64),
                    )
                    nc.vector.memset(va[:, :, 64:65], 1.0)
                return u

            def _oproj_unit(tcx, tt, cc):
                def u():
                    ps = psA.tile([128, 512], f32, tag="psA", name="ops")
                    for hp in range(4):
                        nc.tensor.matmul(
                            ps[:],
                            yt_sb[hp][:, tt * 128:(tt + 1) * 128],
                            wo_sb[hp][:, cc * 512:(cc + 1) * 512],
                            start=(hp == 0),
                            stop=(hp == 3),
                        )
                    osb = outp.tile([128, 512], f32, tag="osb", name="osb")
                    nc.vector.tensor_copy(osb[:], ps[:])
                    nc.sync.dma_start(
                        out[tt * 128:(tt + 1) * 128,
                            cc * 512:(cc + 1) * 512],
                        osb[:],
                    )
                return u

            def _tail_unit(item):
                def u():
                    rec, yr, hp_, qrow_, tcx_ = item
                    bps = psA.tile([64, 512], f32, tag="psA", name="bps")
                    nc.tensor.matmul(
                        bps[:], ones_sb[:, 0:64], rec, start=True, stop=True
                    )
                    nc.vector.tensor_mul(
                        yt_sb[hp_][qrow_, tcx_ * 512:(tcx_ + 1) * 512],
                        yr[0:64, :],
                        bps[:],
                    )
                return u

            pend = []

            for tcx in range(NT):
                if tcx == 0:
                    # prologue: chunk-0 projections must precede attention
                    for jt in range(8):
                        _qkproj_unit(0, jt)()
                    for st in range(4):
                        _vproj_unit(st)()

                # unit list for this chunk: tails of the previous chunk
                # first (they unblock its output projection), then the
                # previous chunk's output projection interleaved with the
                # next chunk's qk projection, then the next chunk's V.
                opq = ([_oproj_unit(tcx - 1, tt, cc)
                        for tt in range(4 * (tcx - 1), 4 * tcx)
                        for cc in range(2)] if tcx > 0 else [])
                qkq = ([_qkproj_unit(tcx + 1, jt) for jt in range(8)]
                       if tcx < NT - 1 else [])
                # lead with qk-projection units: the tails wait on the
                # reciprocal issued moments ago, so give the DVE a head
                # start before the in-order PE hits the tail matmuls
                units = [qkq.pop(0) for _ in range(min(2, len(qkq)))]
                units += [_tail_unit(it) for it in pend]
                pend = []
                while opq or qkq:
                    if qkq:
                        units.append(qkq.pop(0))
                    if opq:
                        units.append(opq.pop(0))
                if tcx < NT - 1:
                    units += [_vproj_unit(st)
                              for st in range(4 * (tcx + 1), 4 * (tcx + 2))]

                nst = 4 * (tcx + 1)
                total_slots = 8 * (tcx + 1)
                slot = 0
                # delay the drain if the first unit is a tail whose
                # reciprocal was just issued (avoids an in-order PE stall)
                skip_slots = 2 if (units and tcx == NT - 1) else 0

                for hp in range(4):
                    ypsA = psY.tile([65, 512], f32, tag="psY", name="ypsA")
                    ypsB = psY.tile([65, 512], f32, tag="psY", name="ypsB")

                    def _avs(items):
                        for st, lo, pt2 in items:
                            nc.tensor.matmul(
                                ypsA[:, lo:512],
                                va_sb[st][:, 2 * hp, :],
                                pt2[:, 0, lo:512],
                                start=(st == 0),
                                stop=(st == nst - 1),
                            )
                            nc.tensor.matmul(
                                ypsB[:, lo:512],
                                va_sb[st][:, 2 * hp + 1, :],
                                pt2[:, 1, lo:512],
                                start=(st == 0),
                                stop=(st == nst - 1),
                            )

                    prev = None
                    for g0 in range(0, nst, 2):
                        cur = []
                        for st in (g0, g0 + 1):
                            if st >= nst:
                                break
                            lo = max(0, st * 128 - tcx * 512)
                            ps2 = psS.tile([128, 2, 512], f32, tag="psS")
                            nc.tensor.matmul(
                                ps2[:, 0, lo:512],
                                kt_sb[hp][0:64, st * 128:(st + 1) * 128],
                                qt_sb[hp][0:64,
                                          tcx * 512 + lo:(tcx + 1) * 512],
                                start=True,
                                stop=True,
                            )
                            nc.tensor.matmul(
                                ps2[:, 1, lo:512],
                                kt_sb[hp][64:128, st * 128:(st + 1) * 128],
                                qt_sb[hp][64:128,
                                          tcx * 512 + lo:(tcx + 1) * 512],
                                start=True,
                                stop=True,
                            )
                            pt2 = ptp.tile([128, 2, 512], f16, tag="pt")
                            nc.scalar.activation(
                                pt2[:, :, lo:512], ps2[:, :, lo:512], AF.Exp
                            )
                            if st * 128 >= tcx * 512:
                                # diagonal tile: zero the upper triangle of
                                # the 128-wide causal window for both heads.
                                # On GPSIMD: the DVE's strict FIFO would park
                                # this behind a 3.3us pair-end reciprocal,
                                # stalling the dependent AV matmuls.
                                nc.gpsimd.tensor_mul(
                                    pt2[:, :, lo:lo + 128],
                                    pt2[:, :, lo:lo + 128],
                                    mask2_sb[:],
                                )
                            cur.append((st, lo, pt2))
                        if prev is not None:
                            _avs(prev)
                        prev = cur
                        # drain filler units into this slot
                        if units and slot >= skip_slots:
                            k = -(-len(units) // max(1, total_slots - slot))
                            for _ in range(min(k, 2, len(units))):
                                units.pop(0)()
                        slot += 1
                    _avs(prev)

                    # evacuate the accumulators to SBUF on ACT so the PSUM
                    # banks free immediately; batch the pair's softmax
                    # denominators on partitions 0 and 64 (32-aligned!) so
                    # one reciprocal serves two heads
                    yrA = stgp.tile([65, 512], f32, tag="yr", bufs=10,
                                    name="yrA")
                    nc.scalar.activation(yrA[:], ypsA[:], AF.Copy)
                    yrB = stgp.tile([65, 512], f32, tag="yr", bufs=10,
                                    name="yrB")
                    nc.scalar.activation(yrB[:], ypsB[:], AF.Copy)
                    den2 = stgp.tile([65, 512], f32, tag="den2", bufs=2,
                                     name="den2")
                    nc.vector.tensor_copy(den2[0:1, :], yrA[64:65, :])
                    nc.vector.tensor_copy(den2[64:65, :], yrB[64:65, :])
                    rec2 = recp.tile([65, 512], f16, tag="rec2", bufs=4)
                    with nc.allow_low_precision(reason="f16 smax recip"):
                        nc.vector.reciprocal(rec2[:], den2[:])
                    recO = recp.tile([1, 512], f16, tag="recO", bufs=4)
                    nc.vector.tensor_copy(recO[:], rec2[64:65, :])
                    tails = [(rec2[0:1, :], yrA, hp, slice(0, 64), tcx),
                             (recO[:], yrB, hp, slice(64, 128), tcx)]
                    if hp < 3:
                        # flush this pair's divides during the next pair's
                        # stream (the reciprocal finishes under it); only
                        # the chunk's last pair carries over
                        units.extend(_tail_unit(it) for it in tails)
                    else:
                        pend.extend(tails)

                # leftover units (shouldn't normally happen)
                while units:
                    units.pop(0)()

            # final chunk: remaining tails (short PE stall) + last oproj
            for it in pend:
                _tail_unit(it)()
            for tt in range(4 * (NT - 1), 4 * NT):
                for cc in range(2):
                    _oproj_unit(NT - 1, tt, cc)()
    return _split_excess_waits(nc, max_waits=1)


def _prep_in_maps(x, W_qkv, b_qkv, W_out):
    f16 = np.float16
    x = np.asarray(x, np.float32)
    W_qkv = np.asarray(W_qkv, np.float32)
    b_qkv = np.asarray(b_qkv, np.float32)
    W_out = np.asarray(W_out, np.float32)

    tri = np.where(
        np.triu(np.ones((128, 128), dtype=bool)), 1.0, 0.0
    ).astype(f16)
    mask2 = np.stack([tri, tri], axis=1)          # [128, 2, 128]
    ones1 = np.ones((1, 128), dtype=f16)
    in_maps = []
    for c in range(8):
        b, hg = divmod(c, 2)
        qs = slice(hg * 512, (hg + 1) * 512)
        ks = slice(E + hg * 512, E + (hg + 1) * 512)
        vs = slice(2 * E + hg * 512, 2 * E + (hg + 1) * 512)
        wqk_c = np.concatenate(
            [W_qkv[:, qs] * 0.125, W_qkv[:, ks]], axis=1
        ).astype(f16)
        bqk_c = np.concatenate(
            [b_qkv[qs] * 0.125, b_qkv[ks]]
        ).astype(np.float32).reshape(8, 128).T.copy()
        in_maps.append({
            "xt": np.ascontiguousarray(x[b].T).astype(f16),
            "wqk": wqk_c,
            "bqk": bqk_c,
            "wv": W_qkv[:, vs].astype(f16),
            "bv": b_qkv[vs].astype(f16).reshape(1, 512),
            "wo": W_out[hg * 512:(hg + 1) * 512, :].astype(f16),
            "mask2": mask2,
            "ones1": ones1,
        })
    return in_maps


def run(x, W_qkv, b_qkv, W_out, b_out, trace=False, **trace_kwargs):
    from concourse.bass_utils import run_bass_kernel_spmd

    if "nc" not in _CACHE:
        _CACHE["nc"] = _build()
    nc = _CACHE["nc"]
    in_maps = _prep_in_maps(x, W_qkv, b_qkv, W_out)
    res = run_bass_kernel_spmd(
        nc, in_maps, list(range(8)), trace=trace, **trace_kwargs
    )
    parts = [r["out"] for r in res.results]
    b_out = np.asarray(b_out, np.float32)
    y = np.stack([parts[2 * b] + parts[2 * b + 1] for b in range(B)]) + b_out
    return y.astype(np.float32), res


def kernel(x, W_qkv, b_qkv, W_out, b_out):
    y, _ = run(x, W_qkv, b_qkv, W_out, b_out, trace=False)
    return y
